# revision 1
# baseline (speedup 1.0000x reference)
"""H2GCN (2-layer GCN with concatenated reps) Trainium2 Bass kernel.

Strategy (8 NeuronCores, nodes sharded):
- Node space relabeled: per-core degree-sorted, padded to NLOC working slots
  per core ("ghosts" double as zero rows for slot padding). Table rows live
  in a block-major layout so the layer-2 table can be produced by NB
  pipelined block-AllGathers that overlap layer-1 aggregation.
- Layer-0 (embed) + layer-1 gather table computed fully replicated on every
  core from a pre-transposed replicated x (kills one 51MB AllGather).
- Aggregation per 512-dst super-tile: dma_gather fetches message rows
  (512B each) from the table; a selection matrix S (one fused tensor_scalar:
  (iota == dstl) * dinv_dst) turns segment-sum into PSUM-accumulated
  matmuls: hT += M_chunk^T @ S_chunk. Output is feature-major so no
  transposes are needed anywhere.
- dma_gather indices are int16, so the table is addressed in 4 quarters
  (< 32768 rows each); slots are grouped by (super-tile, quarter).
- Classifier fused into the layer-2 epilogue; h0 recomputed per tile.
"""

import sys

sys.path.insert(0, "/opt/trn_rl_repo")

import numpy as np

D = 128
D_OUT = 40
NC = 8
P = 128


class Dims:
    def __init__(self, n):
        self.N = n
        self.NLOC_REAL = n // NC
        self.TPC = (self.NLOC_REAL + P - 1) // P          # tiles per core
        self.NLOC = self.TPC * P
        self.NTOT = NC * self.NLOC
        self.NSUP = (self.TPC + 3) // 4
        self.NQ = 4 if self.NTOT >= 4 * P else 1
        self.QR = self.NTOT // self.NQ
        assert self.QR <= 32768, "quarter must fit int16 indexing"
        # allgather blocks: largest NB <= 7 dividing TPC
        self.NB = 1
        for nb in range(7, 0, -1):
            if self.TPC % nb == 0:
                self.NB = nb
                break
        self.TPB = self.TPC // self.NB
        self.BR = self.TPB * P


class Prep:
    """Host-side graph preprocessing: shared instruction schedule plus
    per-core index/metadata arrays."""

    def __init__(self, edge_index: np.ndarray, dims: Dims):
        d = self.d = dims
        N, NLOC, NTOT, NQ, QR, NSUP = d.N, d.NLOC, d.NTOT, d.NQ, d.QR, d.NSUP
        src = edge_index[0].astype(np.int64)
        dst = edge_index[1].astype(np.int64)

        deg = np.bincount(dst, minlength=N).astype(np.int64) + 1
        self.dinv = (1.0 / np.sqrt(deg)).astype(np.float32)

        # per-core degree-sorted relabeling
        w_of_g = np.empty(N, np.int64)
        self.g_of_p = np.full((NC, NLOC), -1, np.int64)
        for c in range(NC):
            g0, g1 = c * d.NLOC_REAL, (c + 1) * d.NLOC_REAL
            order = np.argsort(deg[g0:g1], kind="stable")
            self.g_of_p[c, : d.NLOC_REAL] = g0 + order
            w_of_g[g0 + order] = c * NLOC + np.arange(d.NLOC_REAL)

        # block-major table row for every working id
        w_all = np.arange(NTOT, dtype=np.int64)
        cw, pw = w_all // NLOC, w_all % NLOC
        bw = pw // d.BR
        trow_of_w = bw * (NC * d.BR) + cw * d.BR + (pw - bw * d.BR)

        # edges per core (incl. self-loops), grouped by (super, quarter)
        loops = np.arange(N, dtype=np.int64)
        esrc = np.concatenate([src, loops])
        edst = np.concatenate([dst, loops])
        wsrc = w_of_g[esrc]
        wdst = w_of_g[edst]
        core = wdst // NLOC
        srow = trow_of_w[wsrc]
        equarter = srow // QR
        eqidx = (srow % QR).astype(np.int32)
        elocal = wdst % NLOC
        esup = elocal // 512
        edstl = (elocal - esup * 512).astype(np.float32)
        esd = self.dinv[edst]

        key = (core * NSUP + esup) * NQ + equarter
        cnt = np.bincount(key, minlength=NC * NSUP * NQ).reshape(NC, NSUP, NQ)
        self.nch_sq = ((cnt + P - 1) // P).max(axis=0)   # shared [NSUP, NQ]
        self.ch_off = np.zeros((NSUP, NQ), np.int64)
        run = 0
        for s in range(NSUP):
            for q in range(NQ):
                self.ch_off[s, q] = run
                run += self.nch_sq[s, q]
        self.CH = int(run)
        self.IDXW = int(8 * run)

        self.qidx = np.zeros((NC, P, self.IDXW), np.int16)
        self.dstl = np.full((NC, P, self.CH), -1.0, np.float32)
        self.sd = np.zeros((NC, P, self.CH), np.float32)

        order = np.lexsort((equarter, esup, core))
        o_key = key[order]
        o_qidx = eqidx[order]
        o_dstl = edstl[order]
        o_sd = esd[order]
        bounds = np.searchsorted(o_key, np.arange(NC * NSUP * NQ + 1), "left")
        for c in range(NC):
            for s in range(NSUP):
                for q in range(NQ):
                    k = (c * NSUP + s) * NQ + q
                    lo, hi = bounds[k], bounds[k + 1]
                    n = hi - lo
                    if n == 0:
                        continue
                    ci0 = int(self.ch_off[s, q])
                    nslots = int(self.nch_sq[s, q]) * P
                    i = np.arange(n)
                    self.dstl[c, i % P, ci0 + i // P] = o_dstl[lo:hi]
                    self.sd[c, i % P, ci0 + i // P] = o_sd[lo:hi]
                    ids = np.zeros(nslots, np.int16)
                    ids[:n] = o_qidx[lo:hi]
                    wr = ids.reshape(-1, 16).T          # [16, nslots/16]
                    w8 = np.tile(wr, (8, 1))            # [128, nslots/16]
                    self.qidx[c, :, 8 * ci0 : 8 * ci0 + nslots // 16] = w8

        # trow -> original global node (or -1 for ghosts)
        g_of_w = np.full(NTOT, -1, np.int64)
        for c in range(NC):
            g_of_w[c * NLOC : (c + 1) * NLOC] = self.g_of_p[c]
        self.g_of_trow = np.empty(NTOT, np.int64)
        self.g_of_trow[trow_of_w] = g_of_w
        dinv_trow = np.zeros(NTOT, np.float32)
        real = self.g_of_trow >= 0
        dinv_trow[real] = self.dinv[self.g_of_trow[real]]
        self.dinvw_cols = np.ascontiguousarray(
            dinv_trow.reshape(NTOT // P, P).T
        )

        self.dinvloc_cols = np.zeros((NC, P, d.TPC), np.float32)
        for c in range(NC):
            dl = np.zeros(NLOC, np.float32)
            m = self.g_of_p[c] >= 0
            dl[m] = self.dinv[self.g_of_p[c][m]]
            self.dinvloc_cols[c] = np.ascontiguousarray(dl.reshape(d.TPC, P).T)

    def make_xt(self, x):
        d = self.d
        xt = np.zeros((d.NTOT, D), np.float32)
        real = self.g_of_trow >= 0
        xt[real] = x[self.g_of_trow[real]]
        xt_full = np.ascontiguousarray(xt.T)
        xt_loc = []
        for c in range(NC):
            xl = np.zeros((d.NLOC, D), np.float32)
            m = self.g_of_p[c] >= 0
            xl[m] = x[self.g_of_p[c][m]]
            xt_loc.append(np.ascontiguousarray(xl.T))
        return xt_full, xt_loc


def build_kernel(prep: Prep):
    from concourse import bass, mybir, tile, bacc
    from contextlib import ExitStack

    F32 = mybir.dt.float32
    I16 = mybir.dt.int16
    I32 = mybir.dt.int32
    AF = mybir.ActivationFunctionType
    ALU = mybir.AluOpType

    d = prep.d
    NTOT, NLOC, TPC, NSUP, NQ, QR = d.NTOT, d.NLOC, d.TPC, d.NSUP, d.NQ, d.QR
    nch_sq, ch_off, CH, IDXW = prep.nch_sq, prep.ch_off, prep.CH, prep.IDXW

    nc = bacc.Bacc("TRN2", target_bir_lowering=False)

    xT = nc.declare_dram_parameter("xT", [P, NTOT], F32, isOutput=False)
    xTloc = nc.declare_dram_parameter("xTloc", [P, NLOC], F32, isOutput=False)
    dinvw = nc.declare_dram_parameter("dinvw", [P, NTOT // P], F32, isOutput=False)
    dinvloc = nc.declare_dram_parameter("dinvloc", [P, TPC], F32, isOutput=False)
    idx_all = nc.declare_dram_parameter("idx_all", [P, IDXW], I16, isOutput=False)
    dstl_all = nc.declare_dram_parameter("dstl_all", [P, CH], F32, isOutput=False)
    sd_all = nc.declare_dram_parameter("sd_all", [P, CH], F32, isOutput=False)
    W_embed = nc.declare_dram_parameter("W_embed", [D, D], F32, isOutput=False)
    b_embed = nc.declare_dram_parameter("b_embed", [D, 1], F32, isOutput=False)
    W1 = nc.declare_dram_parameter("W1", [D, D], F32, isOutput=False)
    b1 = nc.declare_dram_parameter("b1", [D, 1], F32, isOutput=False)
    W2 = nc.declare_dram_parameter("W2", [D, D], F32, isOutput=False)
    b2 = nc.declare_dram_parameter("b2", [D, 1], F32, isOutput=False)
    Wc0 = nc.declare_dram_parameter("Wc0", [D, D_OUT], F32, isOutput=False)
    Wc1 = nc.declare_dram_parameter("Wc1", [D, D_OUT], F32, isOutput=False)
    Wc2 = nc.declare_dram_parameter("Wc2", [D, D_OUT], F32, isOutput=False)
    bcls = nc.declare_dram_parameter("bcls", [P, D_OUT], F32, isOutput=False)
    out_p = nc.declare_dram_parameter("out", [NLOC, D_OUT], F32, isOutput=True)
    import os
    stage = int(os.environ.get("KSTAGE", "3"))
    dbg_r = nc.declare_dram_parameter("dbg_r", [NLOC, D], F32, isOutput=True)
    dbg_c = nc.declare_dram_parameter("dbg_c", [P, NLOC], F32, isOutput=True)

    table1 = nc.dram_tensor("table1", [NTOT, D], F32)
    ag_in = nc.dram_tensor("ag_in", [NLOC, D], F32)
    table2 = nc.dram_tensor("table2", [NTOT, D], F32, addr_space="Shared")
    hT1d = nc.dram_tensor("hT1d", [P, NLOC], F32)

    ctx = ExitStack()
    with tile.TileContext(nc) as tc:
        with (
            tc.tile_pool(name="const", bufs=1) as cpool,
            tc.tile_pool(name="xs", bufs=3) as xs_pool,
            tc.tile_pool(name="h0t", bufs=2) as h0t_pool,
            tc.tile_pool(name="g1", bufs=3) as g1_pool,
            tc.tile_pool(name="mbuf", bufs=3) as m_pool,
            tc.tile_pool(name="idxs", bufs=3) as idx_pool,
            tc.tile_pool(name="meta", bufs=2) as meta_pool,
            tc.tile_pool(name="sbuild", bufs=3) as s_pool,
            tc.tile_pool(name="htile", bufs=3) as h_pool,
            tc.tile_pool(name="cls", bufs=3) as cls_pool,
            tc.tile_pool(name="psum_agg", bufs=2, space="PSUM") as pagg,
            tc.tile_pool(name="psum_sm", bufs=3, space="PSUM") as psm,
            tc.tile_pool(name="psum_cls", bufs=2, space="PSUM") as pcls,
        ):
            def load_const(param, shape, dtype=F32):
                t = cpool.tile(shape, dtype, tag=f"c_{param.name}")
                nc.sync.dma_start(out=t[:], in_=param[:])
                return t

            w_embed_sb = load_const(W_embed, [D, D])
            b_embed_sb = load_const(b_embed, [D, 1])
            w1_sb = load_const(W1, [D, D])
            b1_sb = load_const(b1, [D, 1])
            w2_sb = load_const(W2, [D, D])
            b2_sb = load_const(b2, [D, 1])
            wc0_sb = load_const(Wc0, [D, D_OUT])
            wc1_sb = load_const(Wc1, [D, D_OUT])
            wc2_sb = load_const(Wc2, [D, D_OUT])
            bcls_sb = load_const(bcls, [P, D_OUT])
            dinvw_sb = load_const(dinvw, [P, NTOT // P])
            dinvloc_sb = load_const(dinvloc, [P, TPC])

            iota_i = cpool.tile([P, 512], I32)
            nc.gpsimd.iota(iota_i[:], pattern=[[1, 512]], base=0, channel_multiplier=0)
            iota_f = cpool.tile([P, 512], F32)
            nc.vector.tensor_copy(out=iota_f[:], in_=iota_i[:])

            # ---------------- Phase L0: replicated table1 ----------------
            CW = 512
            for chk in range(NTOT // CW):
                r0 = chk * CW
                xt_t = xs_pool.tile([P, CW], F32, tag="xs")
                nc.sync.dma_start(out=xt_t[:], in_=xT[:, r0 : r0 + CW])
                h0_ps = pagg.tile([P, CW], F32, space="PSUM", tag="pagg")
                nc.tensor.matmul(
                    out=h0_ps[:], lhsT=w_embed_sb[:], rhs=xt_t[:],
                    start=True, stop=True,
                )
                h0_t = h0t_pool.tile([P, CW], F32, tag="h0t")
                nc.scalar.activation(
                    out=h0_t[:], in_=h0_ps[:], func=AF.Relu, bias=b_embed_sb[:, :1]
                )
                for sub in range(4):
                    g_ps = psm.tile([P, D], F32, space="PSUM", tag="psm")
                    nc.tensor.matmul(
                        out=g_ps[:],
                        lhsT=h0_t[:, sub * P : (sub + 1) * P], rhs=w1_sb[:],
                        start=True, stop=True,
                    )
                    tile_idx = chk * 4 + sub
                    g1_t = g1_pool.tile([P, D], F32, tag="g1")
                    nc.scalar.activation(
                        out=g1_t[:], in_=g_ps[:], func=AF.Copy,
                        scale=dinvw_sb[:, tile_idx : tile_idx + 1],
                    )
                    nc.sync.dma_start(
                        out=table1[r0 + sub * P : r0 + (sub + 1) * P, :],
                        in_=g1_t[:],
                    )

            if stage == 0:
                nc.sync.dma_start(out=dbg_r[:, :], in_=table1[0:NLOC, :])

            tc.strict_bb_all_engine_barrier()

            # ---------------- aggregation layers ----------------
            def agg_layer(layer):
                table = table1 if layer == 1 else table2
                b_sb = b1_sb if layer == 1 else b2_sb
                for s in range(NSUP):
                    ntile = min(4, TPC - s * 4)
                    c0 = int(ch_off[s, 0])
                    c1 = int(ch_off[s + 1, 0]) if s + 1 < NSUP else CH
                    ncol = c1 - c0
                    dstl_t = meta_pool.tile([P, ncol], F32, tag="dstl")
                    nc.sync.dma_start(out=dstl_t[:], in_=dstl_all[:, c0:c1])
                    sd_t = meta_pool.tile([P, ncol], F32, tag="sd")
                    nc.sync.dma_start(out=sd_t[:], in_=sd_all[:, c0:c1])

                    ps = pagg.tile([P, 512], F32, space="PSUM", tag="pagg")
                    first = True
                    total = int(nch_sq[s].sum())
                    done = 0
                    for q in range(NQ):
                        nch = int(nch_sq[s, q])
                        if nch == 0:
                            continue
                        ciq = int(ch_off[s, q])
                        m_t = m_pool.tile([P, nch, D], F32, tag="m")
                        ix_t = idx_pool.tile([P, 8 * nch], I16, tag="ix")
                        nc.sync.dma_start(
                            out=ix_t[:], in_=idx_all[:, 8 * ciq : 8 * (ciq + nch)]
                        )
                        gmax = int(os.environ.get("KGMAX", "8"))
                        for k0 in range(0, nch, gmax):
                            kn = min(gmax, nch - k0)
                            nc.gpsimd.dma_gather(
                                m_t[:, k0 : k0 + kn, :],
                                table[q * QR : (q + 1) * QR, :],
                                ix_t[:, 8 * k0 : 8 * (k0 + kn)],
                                kn * P, kn * P, D,
                            )
                        for k in range(nch):
                            ci = ciq + k
                            done += 1
                            if int(os.environ.get("KNOS", "0")):
                                continue
                            s_t = s_pool.tile([P, 512], F32, tag="s")
                            nc.vector.tensor_scalar(
                                out=s_t[:], in0=iota_f[:],
                                scalar1=dstl_t[:, ci - c0 : ci - c0 + 1],
                                scalar2=sd_t[:, ci - c0 : ci - c0 + 1],
                                op0=ALU.is_equal, op1=ALU.mult,
                            )
                            if int(os.environ.get("KNOMM", "0")):
                                continue
                            nc.tensor.matmul(
                                out=ps[:], lhsT=m_t[:, k, :], rhs=s_t[:],
                                start=first, stop=(done == total),
                            )
                            first = False
                    skip_epi = int(os.environ.get("KNOS", "0")) or int(os.environ.get("KNOMM", "0"))
                    for tt in range(ntile if not skip_epi else 0):
                        t = s * 4 + tt
                        ht = h_pool.tile([P, D], F32, tag="ht")
                        nc.scalar.activation(
                            out=ht[:], in_=ps[:, tt * P : (tt + 1) * P],
                            func=AF.Relu, bias=b_sb[:, :1],
                        )
                        if layer == 1:
                            nc.sync.dma_start(
                                out=hT1d[:, t * P : (t + 1) * P], in_=ht[:]
                            )
                            g_ps = psm.tile([P, D], F32, space="PSUM", tag="psm")
                            nc.tensor.matmul(
                                out=g_ps[:], lhsT=ht[:], rhs=w2_sb[:],
                                start=True, stop=True,
                            )
                            g2_t = g1_pool.tile([P, D], F32, tag="g2")
                            nc.scalar.activation(
                                out=g2_t[:], in_=g_ps[:], func=AF.Copy,
                                scale=dinvloc_sb[:, t : t + 1],
                            )
                            nc.sync.dma_start(
                                out=ag_in[t * P : (t + 1) * P, :], in_=g2_t[:]
                            )
                        else:
                            xt_t = xs_pool.tile([P, D], F32, tag="xsc")
                            nc.sync.dma_start(
                                out=xt_t[:], in_=xTloc[:, t * P : (t + 1) * P]
                            )
                            h0_ps = psm.tile([P, D], F32, space="PSUM", tag="psm")
                            nc.tensor.matmul(
                                out=h0_ps[:], lhsT=w_embed_sb[:], rhs=xt_t[:],
                                start=True, stop=True,
                            )
                            h0_t = h_pool.tile([P, D], F32, tag="h0c")
                            nc.scalar.activation(
                                out=h0_t[:], in_=h0_ps[:], func=AF.Relu,
                                bias=b_embed_sb[:, :1],
                            )
                            h1_t = h_pool.tile([P, D], F32, tag="h1c")
                            nc.sync.dma_start(
                                out=h1_t[:], in_=hT1d[:, t * P : (t + 1) * P]
                            )
                            o_ps = pcls.tile([P, D_OUT], F32, space="PSUM", tag="pcls")
                            nc.tensor.matmul(
                                out=o_ps[:], lhsT=h0_t[:], rhs=wc0_sb[:],
                                start=True, stop=False,
                            )
                            nc.tensor.matmul(
                                out=o_ps[:], lhsT=h1_t[:], rhs=wc1_sb[:],
                                start=False, stop=False,
                            )
                            nc.tensor.matmul(
                                out=o_ps[:], lhsT=ht[:], rhs=wc2_sb[:],
                                start=False, stop=True,
                            )
                            o_t = cls_pool.tile([P, D_OUT], F32, tag="o")
                            nc.vector.tensor_tensor(
                                out=o_t[:], in0=o_ps[:], in1=bcls_sb[:], op=ALU.add
                            )
                            nc.sync.dma_start(
                                out=out_p[t * P : (t + 1) * P, :], in_=o_t[:]
                            )
                    if layer == 1 and not int(os.environ.get("KNOAG", "0")):
                        tdone = s * 4 + ntile
                        for b in range(d.NB):
                            bend = (b + 1) * d.TPB
                            if bend <= tdone < bend + 4:
                                nc.gpsimd.collective_compute(
                                    "AllGather",
                                    ALU.bypass,
                                    replica_groups=[list(range(NC))],
                                    ins=[ag_in[b * d.BR : (b + 1) * d.BR, :]],
                                    outs=[
                                        table2[
                                            b * NC * d.BR : (b + 1) * NC * d.BR, :
                                        ]
                                    ],
                                )

            if stage >= 1:
                agg_layer(1)
                if stage == 1:
                    nc.sync.dma_start(out=dbg_c[:, :], in_=hT1d[:, :])
                if stage >= 2:
                    tc.strict_bb_all_engine_barrier()
                    if stage == 2:
                        nc.sync.dma_start(out=dbg_r[:, :], in_=table2[0:NLOC, :])
                    if stage >= 3:
                        agg_layer(2)
    ctx.close()
    nc.compile()
    return nc


_CACHE = {}


def run(x, edge_index, W_embed, b_embed, W_conv1, b_conv1, W_conv2, b_conv2,
        W_cls, b_cls, dims: Dims, trace=False):
    from concourse.bass_utils import run_bass_kernel_spmd

    key = dims.N
    if key not in _CACHE:
        prep = Prep(np.asarray(edge_index), dims)
        nck = build_kernel(prep)
        _CACHE[key] = (prep, nck)
    prep, nck = _CACHE[key]

    xt_full, xt_loc = prep.make_xt(np.asarray(x, np.float32))
    bcls_t = np.broadcast_to(
        np.asarray(b_cls, np.float32).reshape(1, D_OUT), (P, D_OUT)
    ).copy()

    in_maps = []
    for c in range(NC):
        in_maps.append(
            {
                "xT": xt_full,
                "xTloc": xt_loc[c],
                "dinvw": prep.dinvw_cols,
                "dinvloc": prep.dinvloc_cols[c],
                "idx_all": prep.qidx[c],
                "dstl_all": prep.dstl[c],
                "sd_all": prep.sd[c],
                "W_embed": np.asarray(W_embed, np.float32),
                "b_embed": np.asarray(b_embed, np.float32).reshape(D, 1),
                "W1": np.asarray(W_conv1, np.float32),
                "b1": np.asarray(b_conv1, np.float32).reshape(D, 1),
                "W2": np.asarray(W_conv2, np.float32),
                "b2": np.asarray(b_conv2, np.float32).reshape(D, 1),
                "Wc0": np.asarray(W_cls[0:D, :], np.float32),
                "Wc1": np.asarray(W_cls[D : 2 * D, :], np.float32),
                "Wc2": np.asarray(W_cls[2 * D : 3 * D, :], np.float32),
                "bcls": bcls_t,
            }
        )

    res = run_bass_kernel_spmd(nck, in_maps, list(range(NC)), trace=trace)

    out = np.empty((dims.N, D_OUT), np.float32)
    for c in range(NC):
        o = res.results[c]["out"]
        m = prep.g_of_p[c] >= 0
        out[prep.g_of_p[c][m]] = o[m]
    return out, res


def kernel(**inputs) -> np.ndarray:
    dims = Dims(100000)
    out, _ = run(
        inputs["x"], inputs["edge_index"], inputs["W_embed"], inputs["b_embed"],
        inputs["W_conv1"], inputs["b_conv1"], inputs["W_conv2"],
        inputs["b_conv2"], inputs["W_cls"], inputs["b_cls"], dims,
    )
    return out



# revision 20
# speedup vs baseline: 1.4302x; 1.4302x over previous
"""H2GCN (2-layer GCN with concatenated reps) Trainium2 Bass kernel, v2.

Strategy (8 NeuronCores, nodes sharded, all-bf16 datapath):
- Node space relabeled per-core (degree-sorted), padded to NLOC=12800 slots.
  Table rows use a block-major layout (NB=4 blocks) so table2 is produced by
  4 pipelined block-AllGathers (block == int16-index quarter, 25600 rows).
- L0: every core computes the full replicated table1 = h0 @ W1 (bf16,
  un-normalized; the GCN norm dinv_src*dinv_dst is folded entirely into the
  per-message S-matrix values). One [128,1024] x-chunk per step.
- Aggregation: per group of G=5 supers (super = 256 dsts) and quarter q, one
  batched dma_gather pulls all message rows (256B each, bf16); each 128-msg
  chunk turns segment-sum into a PSUM matmul via a selection matrix
  S = (iota==dstl)*norm built by one bf16 tensor_scalar (DVE 4x mode).
- L1 runs group-major (dst-major) and fires AllGather block b as soon as the
  covering supers' epilogues (relu -> h1 stash in SBUF -> W2 matmul -> ag_in)
  are done. L2 runs quarter-major, accumulating into an SBUF fp32 acc so each
  quarter's work starts as soon as its AllGather block lands (explicit dep
  edges onto the collective). Classifier is fused per-group after q=3.
"""

import sys

sys.path.insert(0, "/opt/trn_rl_repo")

import numpy as np

D = 128
D_OUT = 40
NC = 8
P = 128
W = 256          # super width (dst slots per psum tile)
G = 5            # supers per gather/psum group
NB = 4           # AllGather blocks (== index quarters)


class Dims:
    def __init__(self, n):
        self.N = n
        self.NLOC_REAL = n // NC          # 12500
        self.NLOC = 12800                 # padded local slots (100 tiles)
        self.TPC = self.NLOC // P         # 100
        self.NTOT = NC * self.NLOC        # 102400
        self.NSUP = self.NLOC // W        # 50
        self.NG = self.NSUP // G          # 10
        self.NQ = 4
        self.QR = self.NTOT // self.NQ    # 25600 (< 32768 for int16 idx)
        self.TPB = self.TPC // NB         # 25 tiles per block
        self.BRL = self.TPB * P           # 3200 local rows per block
        assert NC * self.BRL == self.QR   # block == quarter


class Prep:
    """Host-side graph preprocessing: shared instruction schedule plus
    per-core index/metadata arrays."""

    def __init__(self, edge_index: np.ndarray, dims: Dims):
        d = self.d = dims
        N, NLOC, NTOT, NQ, QR = d.N, d.NLOC, d.NTOT, d.NQ, d.QR
        NSUP, NG = d.NSUP, d.NG
        src = edge_index[0].astype(np.int64)
        dst = edge_index[1].astype(np.int64)

        deg = np.bincount(dst, minlength=N).astype(np.int64) + 1
        self.dinv = (1.0 / np.sqrt(deg)).astype(np.float32)

        # per-core degree-sorted relabeling (ghost slots at the end)
        w_of_g = np.empty(N, np.int64)
        self.g_of_p = np.full((NC, NLOC), -1, np.int64)
        for c in range(NC):
            g0, g1 = c * d.NLOC_REAL, (c + 1) * d.NLOC_REAL
            order = np.argsort(deg[g0:g1], kind="stable")
            self.g_of_p[c, : d.NLOC_REAL] = g0 + order
            w_of_g[g0 + order] = c * NLOC + np.arange(d.NLOC_REAL)

        # block-major table row for every working id
        w_all = np.arange(NTOT, dtype=np.int64)
        cw, pw = w_all // NLOC, w_all % NLOC
        bw = pw // d.BRL
        trow_of_w = bw * QR + cw * d.BRL + (pw - bw * d.BRL)

        # edges per core (incl. self-loops)
        loops = np.arange(N, dtype=np.int64)
        esrc = np.concatenate([src, loops])
        edst = np.concatenate([dst, loops])
        wsrc = w_of_g[esrc]
        wdst = w_of_g[edst]
        core = wdst // NLOC
        srow = trow_of_w[wsrc]
        equarter = srow // QR
        eqidx = (srow % QR).astype(np.int64)
        elocal = wdst % NLOC
        esup = elocal // W
        edstl = (elocal - esup * W).astype(np.float32)
        enorm = (self.dinv[esrc] * self.dinv[edst]).astype(np.float32)

        # shared chunk schedule: canonical order (g, q, s within group)
        key_csq = (core * NSUP + esup) * NQ + equarter
        cnt = np.bincount(key_csq, minlength=NC * NSUP * NQ).reshape(
            NC, NSUP, NQ
        )
        nch_sq = ((cnt.max(axis=0) + P - 1) // P).astype(np.int64)  # [NSUP,NQ]
        self.nch_sq = nch_sq

        # chunk column layout + per-(g,q) gather slices
        self.cells = {}    # (g,q) -> (ci0, totch, [(s_local, ciA, nch)])
        ci_of_cell = np.zeros((NSUP, NQ), np.int64)
        run = 0
        for g in range(NG):
            for q in range(NQ):
                ci0 = run
                lst = []
                for sl in range(G):
                    s = g * G + sl
                    n = int(nch_sq[s, q])
                    ci_of_cell[s, q] = run
                    if n:
                        lst.append((sl, run, n))
                    run += n
                self.cells[(g, q)] = (ci0, run - ci0, lst)
        self.CH = int(run)
        self.IDXW = 8 * self.CH

        # per-super totals for L1 start/stop bookkeeping
        self.ch_per_super = nch_sq.sum(axis=1)  # [NSUP]

        # per-core slot data
        self.qidx = np.zeros((NC, P, self.IDXW), np.int16)
        self.dstl = np.full((NC, P, self.CH), -1.0, np.float32)
        self.sd = np.zeros((NC, P, self.CH), np.float32)

        order = np.lexsort((eqidx, equarter, esup, core))
        o_key = key_csq[order]
        o_qidx = eqidx[order]
        o_dstl = edstl[order]
        o_sd = enorm[order]
        bounds = np.searchsorted(o_key, np.arange(NC * NSUP * NQ + 1), "left")
        for c in range(NC):
            for s in range(NSUP):
                for q in range(NQ):
                    k = (c * NSUP + s) * NQ + q
                    lo, hi = bounds[k], bounds[k + 1]
                    n = hi - lo
                    nch = int(nch_sq[s, q])
                    if nch == 0:
                        assert n == 0
                        continue
                    ci0 = int(ci_of_cell[s, q])
                    nslots = nch * P
                    if n:
                        i = np.arange(n)
                        self.dstl[c, i % P, ci0 + i // P] = o_dstl[lo:hi]
                        self.sd[c, i % P, ci0 + i // P] = o_sd[lo:hi]
                    ids = np.zeros(nslots, np.int16)
                    ids[:n] = o_qidx[lo:hi]
                    wr = ids.reshape(-1, 16).T          # [16, nslots/16]
                    w8 = np.tile(wr, (8, 1))            # [128, nslots/16]
                    self.qidx[c, :, 8 * ci0 : 8 * ci0 + nslots // 16] = w8

        # trow -> original global node (or -1 for ghosts)
        g_of_w = np.full(NTOT, -1, np.int64)
        for c in range(NC):
            g_of_w[c * NLOC : (c + 1) * NLOC] = self.g_of_p[c]
        self.g_of_trow = np.empty(NTOT, np.int64)
        self.g_of_trow[trow_of_w] = g_of_w

    def make_xt(self, x):
        import ml_dtypes

        d = self.d
        bf16 = np.dtype(ml_dtypes.bfloat16)
        xt = np.zeros((d.NTOT, D), np.float32)
        real = self.g_of_trow >= 0
        xt[real] = x[self.g_of_trow[real]]
        xt_full = np.ascontiguousarray(xt.T).astype(bf16)
        xt_loc = []
        for c in range(NC):
            xl = np.zeros((d.NLOC, D), np.float32)
            m = self.g_of_p[c] >= 0
            xl[m] = x[self.g_of_p[c][m]]
            xt_loc.append(np.ascontiguousarray(xl.T).astype(bf16))
        return xt_full, xt_loc


def build_kernel(prep: Prep):
    import os
    from concourse import bass, mybir, tile, bacc
    from concourse.bass import _add_dep_helper

    stage = int(os.environ.get("KSTAGE", "3"))   # 0=L0, 1=+L1, 2=+AG, 3=full
    nodep = int(os.environ.get("KNODEP", "0"))   # 1: barrier instead of cc deps

    F32 = mybir.dt.float32
    BF16 = mybir.dt.bfloat16
    I16 = mybir.dt.int16
    I32 = mybir.dt.int32
    AF = mybir.ActivationFunctionType
    ALU = mybir.AluOpType

    d = prep.d
    NTOT, NLOC, TPC = d.NTOT, d.NLOC, d.TPC
    NSUP, NG, NQ, QR = d.NSUP, d.NG, d.NQ, d.QR
    CH, IDXW = prep.CH, prep.IDXW
    nch_sq = prep.nch_sq

    nc = bacc.Bacc("TRN2", target_bir_lowering=False)

    xT = nc.declare_dram_parameter("xT", [P, NTOT], BF16, isOutput=False)
    xTloc = nc.declare_dram_parameter("xTloc", [P, NLOC], BF16, isOutput=False)
    idx_all = nc.declare_dram_parameter("idx_all", [P, IDXW], I16, isOutput=False)
    dstl_all = nc.declare_dram_parameter("dstl_all", [P, CH], F32, isOutput=False)
    sd_all = nc.declare_dram_parameter("sd_all", [P, CH], F32, isOutput=False)
    W_embed = nc.declare_dram_parameter("W_embed", [D, D], BF16, isOutput=False)
    b_embed = nc.declare_dram_parameter("b_embed", [D, 1], F32, isOutput=False)
    W1 = nc.declare_dram_parameter("W1", [D, D], BF16, isOutput=False)
    b1 = nc.declare_dram_parameter("b1", [D, 1], F32, isOutput=False)
    W2 = nc.declare_dram_parameter("W2", [D, D], BF16, isOutput=False)
    b2 = nc.declare_dram_parameter("b2", [D, 1], F32, isOutput=False)
    Wc0 = nc.declare_dram_parameter("Wc0", [D, D_OUT], BF16, isOutput=False)
    Wc1 = nc.declare_dram_parameter("Wc1", [D, D_OUT], BF16, isOutput=False)
    Wc2 = nc.declare_dram_parameter("Wc2", [D, D_OUT], BF16, isOutput=False)
    bcls = nc.declare_dram_parameter("bcls", [P, D_OUT], F32, isOutput=False)
    out_p = nc.declare_dram_parameter("out", [NLOC, D_OUT], F32, isOutput=True)

    table1 = nc.dram_tensor("table1", [NTOT, D], BF16)
    ag_in = nc.dram_tensor("ag_in", [NLOC, D], BF16)
    table2 = nc.dram_tensor("table2", [NTOT, D], BF16, addr_space="Shared")
    if stage < 3:
        dbg = nc.declare_dram_parameter("dbg", [NLOC, D], BF16, isOutput=True)

    # AG block b fires after L1 finishes group ag_after[b]
    ag_after = {}
    for b in range(NB):
        need_sup = -(-(d.TPB * (b + 1)) // 2) - 1    # last super covering block
        ag_after[(need_sup // G)] = b

    with tile.TileContext(nc) as tc:
        with (
            tc.tile_pool(name="const", bufs=1) as cpool,
            tc.tile_pool(name="xs", bufs=3) as xs_pool,
            tc.tile_pool(name="h0t", bufs=2) as h0t_pool,
            tc.tile_pool(name="g1", bufs=2) as g1_pool,
            tc.tile_pool(name="mbuf", bufs=2) as m_pool,
            tc.tile_pool(name="sbuild", bufs=4) as s_pool,
            tc.tile_pool(name="htile", bufs=3) as h_pool,
            tc.tile_pool(name="cls", bufs=3) as cls_pool,
            tc.tile_pool(name="psum_big", bufs=2, space="PSUM") as p_big,
            tc.tile_pool(name="psum_small", bufs=2, space="PSUM") as p_small,
        ):
            def load_const(param, shape, dtype=F32):
                t = cpool.tile(shape, dtype, tag=f"c_{param.name}")
                nc.sync.dma_start(out=t[:], in_=param[:])
                return t

            we_sb = load_const(W_embed, [D, D], BF16)
            be_sb = load_const(b_embed, [D, 1])
            w1_sb = load_const(W1, [D, D], BF16)
            b1_sb = load_const(b1, [D, 1])
            w2_sb = load_const(W2, [D, D], BF16)
            b2_sb = load_const(b2, [D, 1])
            wc0_sb = load_const(Wc0, [D, D_OUT], BF16)
            wc1_sb = load_const(Wc1, [D, D_OUT], BF16)
            wc2_sb = load_const(Wc2, [D, D_OUT], BF16)
            bcls_sb = load_const(bcls, [P, D_OUT])
            idx_sb = load_const(idx_all, [P, IDXW], I16)
            dstl_sb = load_const(dstl_all, [P, CH])
            sd_sb = load_const(sd_all, [P, CH])

            iota_i = cpool.tile([P, W], I32, tag="iota_i")
            nc.gpsimd.iota(iota_i[:], pattern=[[1, W]], base=0, channel_multiplier=0)
            iota_f = cpool.tile([P, W], F32, tag="iota_f")
            nc.vector.tensor_copy(out=iota_f[:], in_=iota_i[:])
            iota_bf = cpool.tile([P, W], BF16, tag="iota_bf")
            nc.vector.tensor_copy(out=iota_bf[:], in_=iota_f[:])

            # persistent SBUF: h1 (feature-major) and L2 accumulator
            h1st = cpool.tile([P, NLOC], BF16, tag="h1st")
            nc.vector.memset(h1st[:], 0.0)
            acc2 = cpool.tile([P, NLOC], F32, tag="acc2")
            nc.vector.memset(acc2[:], 0.0)

            # ---------------- Phase L0: replicated table1 ----------------
            CW = 1024
            with nc.named_scope("L0"):
                for chk in range(NTOT // CW):
                    r0 = chk * CW
                    xt_t = xs_pool.tile([P, CW], BF16, tag="xs")
                    nc.sync.dma_start(out=xt_t[:], in_=xT[:, r0 : r0 + CW])
                    g1_ps = p_big.tile([P, 1536], F32, space="PSUM", tag="big")
                    for half in range(2):
                        h0_ps = p_small.tile([P, 512], F32, space="PSUM", tag="small")
                        nc.tensor.matmul(
                            out=h0_ps[:],
                            lhsT=we_sb[:],
                            rhs=xt_t[:, half * 512 : (half + 1) * 512],
                            start=True, stop=True,
                        )
                        h0_t = h0t_pool.tile([P, 512], BF16, tag="h0t")
                        nc.scalar.activation(
                            out=h0_t[:], in_=h0_ps[:], func=AF.Relu,
                            bias=be_sb[:, :1],
                        )
                        for sub in range(4):
                            j = half * 4 + sub
                            nc.tensor.matmul(
                                out=g1_ps[:, j * P : (j + 1) * P],
                                lhsT=h0_t[:, sub * P : (sub + 1) * P],
                                rhs=w1_sb[:],
                                start=True, stop=True,
                            )
                    g1_t = g1_pool.tile([P, CW], BF16, tag="g1")
                    nc.vector.tensor_copy(out=g1_t[:], in_=g1_ps[:, 0:CW])
                    # one write: row n=r0+j*128+p  <-  g1_t[p, j*128: ]
                    t_ap = table1[r0 : r0 + CW, :].rearrange(
                        "(j p) d -> p j d", p=P
                    )
                    nc.sync.dma_start(
                        out=t_ap, in_=g1_t[:].rearrange("p (j d) -> p j d", d=D)
                    )

            if stage == 0:
                nc.sync.dma_start(out=dbg[:, :], in_=table1[0:NLOC, :])

            tc.strict_bb_all_engine_barrier()

            # ---------------- aggregation machinery ----------------
            KG = int(os.environ.get("KGMAX", "8"))

            def do_group_chunks(table, g, q, ps, started, remaining, cc_dep,
                                bank_started):
                """Gather + S-matmuls for all cells (s in group g, quarter q).
                started/remaining: per-s_local accumulation bookkeeping.
                bank_started: per-PSUM-bank flag — matmul start=True clears
                has_written for the WHOLE bank, so only the first chunk
                touching each bank may carry start=True."""
                ci0, totch, lst = prep.cells[(g, q)]
                if totch == 0:
                    return
                sl_of = np.empty(totch, np.int64)
                for (sl, ciA, nchn) in lst:
                    sl_of[ciA - ci0 : ciA - ci0 + nchn] = sl
                for w0 in range(0, totch, KG):
                    wn = min(KG, totch - w0)
                    m_t = m_pool.tile([P, wn, D], BF16, tag="m")
                    gi = nc.gpsimd.dma_gather(
                        m_t[:, :, :],
                        table[q * QR : (q + 1) * QR, :],
                        idx_sb[:, 8 * (ci0 + w0) : 8 * (ci0 + w0 + wn)],
                        wn * P, wn * P, D,
                    )
                    if cc_dep is not None:
                        _add_dep_helper(
                            gi.ins, cc_dep.ins, sync=True,
                            reason="gather waits for AllGather block",
                        )
                    for k in range(wn):
                        ci = ci0 + w0 + k
                        sl = int(sl_of[w0 + k])
                        s_t = s_pool.tile([P, W], BF16, tag="s")
                        nc.vector.tensor_scalar(
                            out=s_t[:], in0=iota_bf[:],
                            scalar1=dstl_sb[:, ci : ci + 1],
                            scalar2=sd_sb[:, ci : ci + 1],
                            op0=ALU.is_equal, op1=ALU.mult,
                        )
                        remaining[sl] -= 1
                        bank = sl // 2
                        nc.tensor.matmul(
                            out=ps[:, sl * W : (sl + 1) * W],
                            lhsT=m_t[:, k, :],
                            rhs=s_t[:],
                            start=not bank_started[bank],
                            stop=remaining[sl] == 0,
                            skip_group_check=True,
                        )
                        bank_started[bank] = True
                        started[sl] = True

            # ---------------- Phase L1 (dst-major) + progressive AG -------
            cc_insts = []
            with nc.named_scope("L1"):
                for g in range(NG if stage >= 1 else 0):
                    ps = p_big.tile([P, 1536], F32, space="PSUM", tag="big")
                    started = [False] * G
                    bank_started = [False] * 3
                    remaining = [int(prep.ch_per_super[g * G + sl]) for sl in range(G)]
                    for q in range(NQ):
                        do_group_chunks(table1, g, q, ps, started, remaining,
                                        None, bank_started)
                    # epilogue per super: relu -> h1 stash; W2 -> ag_in
                    for sl in range(G):
                        s = g * G + sl
                        if not started[sl]:
                            continue
                        for half in range(2):
                            t = s * 2 + half
                            nc.scalar.activation(
                                out=h1st[:, t * P : (t + 1) * P],
                                in_=ps[:, sl * W + half * P : sl * W + (half + 1) * P],
                                func=AF.Relu, bias=b1_sb[:, :1],
                            )
                        ps2 = p_small.tile([P, 512], F32, space="PSUM", tag="small")
                        for half in range(2):
                            nc.tensor.matmul(
                                out=ps2[:, half * P : (half + 1) * P],
                                lhsT=h1st[:, (s * 2 + half) * P : (s * 2 + half + 1) * P],
                                rhs=w2_sb[:],
                                start=True, stop=True,
                            )
                        g2_t = g1_pool.tile([P, W], BF16, tag="g2")
                        nc.vector.tensor_copy(out=g2_t[:], in_=ps2[:, 0:W])
                        o_ap = ag_in[s * W : (s + 1) * W, :].rearrange(
                            "(u p) d -> p u d", p=P
                        )
                        nc.sync.dma_start(
                            out=o_ap,
                            in_=g2_t[:].rearrange("p (u d) -> p u d", d=D),
                        )
                    if g in ag_after and stage >= 2:
                        b = ag_after[g]
                        cc = nc.gpsimd.collective_compute(
                            "AllGather",
                            ALU.bypass,
                            replica_groups=[list(range(NC))],
                            ins=[ag_in[b * d.BRL : (b + 1) * d.BRL, :]],
                            outs=[table2[b * QR : (b + 1) * QR, :]],
                        )
                        cc_insts.append(cc)

            if stage == 1:
                nc.sync.dma_start(out=dbg[:, :], in_=ag_in[:, :])
            if stage == 2:
                tc.strict_bb_all_engine_barrier()
                nc.sync.dma_start(out=dbg[:, :], in_=table2[0:NLOC, :])
            if nodep and stage >= 3:
                tc.strict_bb_all_engine_barrier()

            # ---------------- Phase L2 (quarter-major) + fused classifier -
            with nc.named_scope("L2"):
                for q in range(NQ if stage >= 3 else 0):
                    for g in range(NG):
                        ps = p_big.tile([P, 1536], F32, space="PSUM", tag="big")
                        started = [False] * G
                        bank_started = [False] * 3
                        remaining = [int(nch_sq[g * G + sl, q]) for sl in range(G)]
                        do_group_chunks(
                            table2, g, q, ps, started, remaining,
                            None if nodep else cc_insts[q], bank_started,
                        )
                        # merge written supers into acc (contiguous runs)
                        sl = 0
                        while sl < G:
                            if not started[sl]:
                                sl += 1
                                continue
                            sl2 = sl
                            while sl2 + 1 < G and started[sl2 + 1]:
                                sl2 += 1
                            a0 = (g * G + sl) * W
                            a1 = (g * G + sl2 + 1) * W
                            nc.vector.tensor_tensor(
                                out=acc2[:, a0:a1],
                                in0=ps[:, sl * W : (sl2 + 1) * W],
                                in1=acc2[:, a0:a1],
                                op=ALU.add,
                            )
                            sl = sl2 + 1
                        if q == NQ - 1:
                            # classifier for this group's tiles
                            for t in range(g * G * 2, (g + 1) * G * 2):
                                h2_t = h_pool.tile([P, P], BF16, tag="h2")
                                nc.vector.tensor_scalar(
                                    out=h2_t[:],
                                    in0=acc2[:, t * P : (t + 1) * P],
                                    scalar1=b2_sb[:, :1],
                                    scalar2=0.0,
                                    op0=ALU.add, op1=ALU.max,
                                )
                                xl_t = xs_pool.tile([P, P], BF16, tag="xl")
                                nc.sync.dma_start(
                                    out=xl_t[:],
                                    in_=xTloc[:, t * P : (t + 1) * P],
                                )
                                h0_ps = p_small.tile(
                                    [P, 512], F32, space="PSUM", tag="small"
                                )
                                nc.tensor.matmul(
                                    out=h0_ps[:, 0:P], lhsT=we_sb[:], rhs=xl_t[:],
                                    start=True, stop=True,
                                )
                                h0_t = h_pool.tile([P, P], BF16, tag="h0c")
                                nc.scalar.activation(
                                    out=h0_t[:], in_=h0_ps[:, 0:P], func=AF.Relu,
                                    bias=be_sb[:, :1],
                                )
                                o_ps = p_small.tile(
                                    [P, 512], F32, space="PSUM", tag="small"
                                )
                                nc.tensor.matmul(
                                    out=o_ps[:, 0:D_OUT], lhsT=h0_t[:], rhs=wc0_sb[:],
                                    start=True, stop=False,
                                )
                                nc.tensor.matmul(
                                    out=o_ps[:, 0:D_OUT],
                                    lhsT=h1st[:, t * P : (t + 1) * P],
                                    rhs=wc1_sb[:],
                                    start=False, stop=False,
                                )
                                nc.tensor.matmul(
                                    out=o_ps[:, 0:D_OUT], lhsT=h2_t[:], rhs=wc2_sb[:],
                                    start=False, stop=True,
                                )
                                o_t = cls_pool.tile([P, D_OUT], F32, tag="o")
                                nc.vector.tensor_tensor(
                                    out=o_t[:], in0=o_ps[:, 0:D_OUT], in1=bcls_sb[:],
                                    op=ALU.add,
                                )
                                nc.sync.dma_start(
                                    out=out_p[t * P : (t + 1) * P, :], in_=o_t[:]
                                )
    nc.compile()
    return nc


_CACHE = {}


def run(x, edge_index, W_embed, b_embed, W_conv1, b_conv1, W_conv2, b_conv2,
        W_cls, b_cls, dims: Dims, trace=False, tmpdir=None):
    import ml_dtypes
    from concourse.bass_utils import run_bass_kernel_spmd

    import os
    bf16 = np.dtype(ml_dtypes.bfloat16)
    key = (dims.N, os.environ.get("KSTAGE", "3"), os.environ.get("KNODEP", "0"),
           os.environ.get("KGMAX", "8"))
    if key not in _CACHE:
        prep = Prep(np.asarray(edge_index), dims)
        nck = build_kernel(prep)
        _CACHE[key] = (prep, nck)
    prep, nck = _CACHE[key]

    xt_full, xt_loc = prep.make_xt(np.asarray(x, np.float32))
    bcls_t = np.broadcast_to(
        np.asarray(b_cls, np.float32).reshape(1, D_OUT), (P, D_OUT)
    ).copy()

    in_maps = []
    for c in range(NC):
        in_maps.append(
            {
                "xT": xt_full,
                "xTloc": xt_loc[c],
                "idx_all": prep.qidx[c],
                "dstl_all": prep.dstl[c],
                "sd_all": prep.sd[c],
                "W_embed": np.asarray(W_embed, np.float32).astype(bf16),
                "b_embed": np.asarray(b_embed, np.float32).reshape(D, 1),
                "W1": np.asarray(W_conv1, np.float32).astype(bf16),
                "b1": np.asarray(b_conv1, np.float32).reshape(D, 1),
                "W2": np.asarray(W_conv2, np.float32).astype(bf16),
                "b2": np.asarray(b_conv2, np.float32).reshape(D, 1),
                "Wc0": np.asarray(W_cls[0:D, :], np.float32).astype(bf16),
                "Wc1": np.asarray(W_cls[D : 2 * D, :], np.float32).astype(bf16),
                "Wc2": np.asarray(W_cls[2 * D : 3 * D, :], np.float32).astype(bf16),
                "bcls": bcls_t,
            }
        )

    res = run_bass_kernel_spmd(
        nck, in_maps, list(range(NC)), trace=trace, tmpdir=tmpdir
    )

    out = np.empty((dims.N, D_OUT), np.float32)
    for c in range(NC):
        o = res.results[c]["out"]
        m = prep.g_of_p[c] >= 0
        out[prep.g_of_p[c][m]] = o[m]
    return out, res


def kernel(**inputs) -> np.ndarray:
    dims = Dims(100000)
    out, _ = run(
        inputs["x"], inputs["edge_index"], inputs["W_embed"], inputs["b_embed"],
        inputs["W_conv1"], inputs["b_conv1"], inputs["W_conv2"],
        inputs["b_conv2"], inputs["W_cls"], inputs["b_cls"], dims,
    )
    return out


# revision 23
# speedup vs baseline: 1.4790x; 1.0341x over previous
"""H2GCN (2-layer GCN with concatenated reps) Trainium2 Bass kernel, v2.

Strategy (8 NeuronCores, nodes sharded, all-bf16 datapath):
- Node space relabeled per-core (degree-sorted), padded to NLOC=12800 slots.
  Table rows use a block-major layout (NB=4 blocks) so table2 is produced by
  4 pipelined block-AllGathers (block == int16-index quarter, 25600 rows).
- L0: every core computes the full replicated table1 = h0 @ W1 (bf16,
  un-normalized; the GCN norm dinv_src*dinv_dst is folded entirely into the
  per-message S-matrix values). One [128,1024] x-chunk per step.
- Aggregation: per group of G=5 supers (super = 256 dsts) and quarter q, one
  batched dma_gather pulls all message rows (256B each, bf16); each 128-msg
  chunk turns segment-sum into a PSUM matmul via a selection matrix
  S = (iota==dstl)*norm built by one bf16 tensor_scalar (DVE 4x mode).
- L1 runs group-major (dst-major) and fires AllGather block b as soon as the
  covering supers' epilogues (relu -> h1 stash in SBUF -> W2 matmul -> ag_in)
  are done. L2 runs quarter-major, accumulating into an SBUF fp32 acc so each
  quarter's work starts as soon as its AllGather block lands (explicit dep
  edges onto the collective). Classifier is fused per-group after q=3.
"""

import sys

sys.path.insert(0, "/opt/trn_rl_repo")

import numpy as np

D = 128
D_OUT = 40
NC = 8
P = 128
W = 256          # super width (dst slots per psum tile)
G = 5            # supers per gather/psum group
NB = 4           # AllGather blocks (== index quarters)


class Dims:
    def __init__(self, n):
        self.N = n
        self.NLOC_REAL = n // NC          # 12500
        self.NLOC = 12800                 # padded local slots (100 tiles)
        self.TPC = self.NLOC // P         # 100
        self.NTOT = NC * self.NLOC        # 102400
        self.NSUP = self.NLOC // W        # 50
        self.NG = self.NSUP // G          # 10
        self.NQ = 4
        self.QR = self.NTOT // self.NQ    # 25600 (< 32768 for int16 idx)
        self.TPB = self.TPC // NB         # 25 tiles per block
        self.BRL = self.TPB * P           # 3200 local rows per block
        assert NC * self.BRL == self.QR   # block == quarter


class Prep:
    """Host-side graph preprocessing: shared instruction schedule plus
    per-core index/metadata arrays."""

    def __init__(self, edge_index: np.ndarray, dims: Dims):
        d = self.d = dims
        N, NLOC, NTOT, NQ, QR = d.N, d.NLOC, d.NTOT, d.NQ, d.QR
        NSUP, NG = d.NSUP, d.NG
        src = edge_index[0].astype(np.int64)
        dst = edge_index[1].astype(np.int64)

        deg = np.bincount(dst, minlength=N).astype(np.int64) + 1
        self.dinv = (1.0 / np.sqrt(deg)).astype(np.float32)

        # per-core degree-sorted relabeling (ghost slots at the end)
        w_of_g = np.empty(N, np.int64)
        self.g_of_p = np.full((NC, NLOC), -1, np.int64)
        for c in range(NC):
            g0, g1 = c * d.NLOC_REAL, (c + 1) * d.NLOC_REAL
            order = np.argsort(deg[g0:g1], kind="stable")
            self.g_of_p[c, : d.NLOC_REAL] = g0 + order
            w_of_g[g0 + order] = c * NLOC + np.arange(d.NLOC_REAL)

        # block-major table row for every working id
        w_all = np.arange(NTOT, dtype=np.int64)
        cw, pw = w_all // NLOC, w_all % NLOC
        bw = pw // d.BRL
        trow_of_w = bw * QR + cw * d.BRL + (pw - bw * d.BRL)

        # edges per core (incl. self-loops)
        loops = np.arange(N, dtype=np.int64)
        esrc = np.concatenate([src, loops])
        edst = np.concatenate([dst, loops])
        wsrc = w_of_g[esrc]
        wdst = w_of_g[edst]
        core = wdst // NLOC
        srow = trow_of_w[wsrc]
        equarter = srow // QR
        eqidx = (srow % QR).astype(np.int64)
        elocal = wdst % NLOC
        esup = elocal // W
        edstl = (elocal - esup * W).astype(np.float32)
        enorm = (self.dinv[esrc] * self.dinv[edst]).astype(np.float32)

        # shared chunk schedule: canonical order (g, q, s within group)
        key_csq = (core * NSUP + esup) * NQ + equarter
        cnt = np.bincount(key_csq, minlength=NC * NSUP * NQ).reshape(
            NC, NSUP, NQ
        )
        nch_sq = ((cnt.max(axis=0) + P - 1) // P).astype(np.int64)  # [NSUP,NQ]
        self.nch_sq = nch_sq

        # chunk column layout + per-(g,q) gather slices
        self.cells = {}    # (g,q) -> (ci0, totch, [(s_local, ciA, nch)])
        ci_of_cell = np.zeros((NSUP, NQ), np.int64)
        run = 0
        for g in range(NG):
            for q in range(NQ):
                ci0 = run
                lst = []
                for sl in range(G):
                    s = g * G + sl
                    n = int(nch_sq[s, q])
                    ci_of_cell[s, q] = run
                    if n:
                        lst.append((sl, run, n))
                    run += n
                self.cells[(g, q)] = (ci0, run - ci0, lst)
        self.CH = int(run)
        self.IDXW = 8 * self.CH

        # per-super totals for L1 start/stop bookkeeping
        self.ch_per_super = nch_sq.sum(axis=1)  # [NSUP]

        # per-core slot data
        self.qidx = np.zeros((NC, P, self.IDXW), np.int16)
        self.dstl = np.full((NC, P, self.CH), -1.0, np.float32)
        self.sd = np.zeros((NC, P, self.CH), np.float32)

        order = np.lexsort((eqidx, equarter, esup, core))
        o_key = key_csq[order]
        o_qidx = eqidx[order]
        o_dstl = edstl[order]
        o_sd = enorm[order]
        bounds = np.searchsorted(o_key, np.arange(NC * NSUP * NQ + 1), "left")
        for c in range(NC):
            for s in range(NSUP):
                for q in range(NQ):
                    k = (c * NSUP + s) * NQ + q
                    lo, hi = bounds[k], bounds[k + 1]
                    n = hi - lo
                    nch = int(nch_sq[s, q])
                    if nch == 0:
                        assert n == 0
                        continue
                    ci0 = int(ci_of_cell[s, q])
                    nslots = nch * P
                    if n:
                        i = np.arange(n)
                        self.dstl[c, i % P, ci0 + i // P] = o_dstl[lo:hi].astype(self.dstl.dtype)
                        self.sd[c, i % P, ci0 + i // P] = o_sd[lo:hi].astype(self.sd.dtype)
                    ids = np.zeros(nslots, np.int16)
                    ids[:n] = o_qidx[lo:hi]
                    wr = ids.reshape(-1, 16).T          # [16, nslots/16]
                    w8 = np.tile(wr, (8, 1))            # [128, nslots/16]
                    self.qidx[c, :, 8 * ci0 : 8 * ci0 + nslots // 16] = w8

        # trow -> original global node (or -1 for ghosts)
        g_of_w = np.full(NTOT, -1, np.int64)
        for c in range(NC):
            g_of_w[c * NLOC : (c + 1) * NLOC] = self.g_of_p[c]
        self.g_of_trow = np.empty(NTOT, np.int64)
        self.g_of_trow[trow_of_w] = g_of_w

    def make_xt(self, x):
        import ml_dtypes

        d = self.d
        bf16 = np.dtype(ml_dtypes.bfloat16)
        xt = np.zeros((d.NTOT, D), np.float32)
        real = self.g_of_trow >= 0
        xt[real] = x[self.g_of_trow[real]]
        xt_full = np.ascontiguousarray(xt.T).astype(bf16)
        xt_loc = []
        for c in range(NC):
            xl = np.zeros((d.NLOC, D), np.float32)
            m = self.g_of_p[c] >= 0
            xl[m] = x[self.g_of_p[c][m]]
            xt_loc.append(np.ascontiguousarray(xl.T).astype(bf16))
        return xt_full, xt_loc


def build_kernel(prep: Prep):
    import os
    from concourse import bass, mybir, tile, bacc
    from concourse.bass import _add_dep_helper

    stage = int(os.environ.get("KSTAGE", "3"))   # 0=L0, 1=+L1, 2=+AG, 3=full
    nodep = int(os.environ.get("KNODEP", "0"))   # 1: barrier instead of cc deps

    F32 = mybir.dt.float32
    BF16 = mybir.dt.bfloat16
    I16 = mybir.dt.int16
    I32 = mybir.dt.int32
    AF = mybir.ActivationFunctionType
    ALU = mybir.AluOpType

    d = prep.d
    NTOT, NLOC, TPC = d.NTOT, d.NLOC, d.TPC
    NSUP, NG, NQ, QR = d.NSUP, d.NG, d.NQ, d.QR
    CH, IDXW = prep.CH, prep.IDXW
    nch_sq = prep.nch_sq

    nc = bacc.Bacc("TRN2", target_bir_lowering=False,
               num_swdge_queues=int(os.environ.get("KSWQ", "2")),
               dynamic_dma_scratch_size=int(os.environ.get("KSCRATCH", "49152")))

    xT = nc.declare_dram_parameter("xT", [P, NTOT], BF16, isOutput=False)
    xTloc = nc.declare_dram_parameter("xTloc", [P, NLOC], BF16, isOutput=False)
    idx_all = nc.declare_dram_parameter("idx_all", [P, IDXW], I16, isOutput=False)
    dstl_all = nc.declare_dram_parameter("dstl_all", [P, CH], F32, isOutput=False)
    sd_all = nc.declare_dram_parameter("sd_all", [P, CH], F32, isOutput=False)
    W_embed = nc.declare_dram_parameter("W_embed", [D, D], BF16, isOutput=False)
    b_embed = nc.declare_dram_parameter("b_embed", [D, 1], F32, isOutput=False)
    W1 = nc.declare_dram_parameter("W1", [D, D], BF16, isOutput=False)
    b1 = nc.declare_dram_parameter("b1", [D, 1], F32, isOutput=False)
    W2 = nc.declare_dram_parameter("W2", [D, D], BF16, isOutput=False)
    b2 = nc.declare_dram_parameter("b2", [D, 1], F32, isOutput=False)
    Wc0 = nc.declare_dram_parameter("Wc0", [D, D_OUT], BF16, isOutput=False)
    Wc1 = nc.declare_dram_parameter("Wc1", [D, D_OUT], BF16, isOutput=False)
    Wc2 = nc.declare_dram_parameter("Wc2", [D, D_OUT], BF16, isOutput=False)
    bcls = nc.declare_dram_parameter("bcls", [P, D_OUT], F32, isOutput=False)
    out_p = nc.declare_dram_parameter("out", [NLOC, D_OUT], F32, isOutput=True)

    table1 = nc.dram_tensor("table1", [NTOT, D], BF16)
    ag_in = nc.dram_tensor("ag_in", [NLOC, D], BF16)
    table2 = nc.dram_tensor("table2", [NTOT, D], BF16, addr_space="Shared")
    if stage < 3:
        dbg = nc.declare_dram_parameter("dbg", [NLOC, D], BF16, isOutput=True)

    # AG block b fires after L1 finishes group ag_after[b]
    ag_after = {}
    for b in range(NB):
        need_sup = -(-(d.TPB * (b + 1)) // 2) - 1    # last super covering block
        ag_after[(need_sup // G)] = b

    with tile.TileContext(nc) as tc:
        with (
            tc.tile_pool(name="const", bufs=1) as cpool,
            tc.tile_pool(name="xs", bufs=3) as xs_pool,
            tc.tile_pool(name="h0t", bufs=2) as h0t_pool,
            tc.tile_pool(name="g1", bufs=2) as g1_pool,
            tc.tile_pool(name="mbuf", bufs=2) as m_pool,
            tc.tile_pool(name="sbuild", bufs=4) as s_pool,
            tc.tile_pool(name="htile", bufs=3) as h_pool,
            tc.tile_pool(name="cls", bufs=3) as cls_pool,
            tc.tile_pool(name="psum_big", bufs=2, space="PSUM") as p_big,
            tc.tile_pool(name="psum_small", bufs=2, space="PSUM") as p_small,
        ):
            def load_const(param, shape, dtype=F32):
                t = cpool.tile(shape, dtype, tag=f"c_{param.name}")
                nc.sync.dma_start(out=t[:], in_=param[:])
                return t

            we_sb = load_const(W_embed, [D, D], BF16)
            be_sb = load_const(b_embed, [D, 1])
            w1_sb = load_const(W1, [D, D], BF16)
            b1_sb = load_const(b1, [D, 1])
            w2_sb = load_const(W2, [D, D], BF16)
            b2_sb = load_const(b2, [D, 1])
            wc0_sb = load_const(Wc0, [D, D_OUT], BF16)
            wc1_sb = load_const(Wc1, [D, D_OUT], BF16)
            wc2_sb = load_const(Wc2, [D, D_OUT], BF16)
            bcls_sb = load_const(bcls, [P, D_OUT])
            idx_sb = load_const(idx_all, [P, IDXW], I16)
            dstl_sb = load_const(dstl_all, [P, CH])
            sd_sb = load_const(sd_all, [P, CH])

            iota_i = cpool.tile([P, W], I32, tag="iota_i")
            nc.gpsimd.iota(iota_i[:], pattern=[[1, W]], base=0, channel_multiplier=0)
            iota_f = cpool.tile([P, W], F32, tag="iota_f")
            nc.vector.tensor_copy(out=iota_f[:], in_=iota_i[:])
            iota_bf = cpool.tile([P, W], BF16, tag="iota_bf")
            nc.vector.tensor_copy(out=iota_bf[:], in_=iota_f[:])

            # persistent SBUF: h1 (feature-major) and L2 accumulator
            h1st = cpool.tile([P, NLOC], BF16, tag="h1st")
            nc.vector.memset(h1st[:], 0.0)
            acc2 = cpool.tile([P, NLOC], F32, tag="acc2")
            nc.vector.memset(acc2[:], 0.0)

            # ---------------- Phase L0: replicated table1 ----------------
            CW = 1024
            with nc.named_scope("L0"):
                for chk in range(NTOT // CW):
                    r0 = chk * CW
                    xt_t = xs_pool.tile([P, CW], BF16, tag="xs")
                    nc.sync.dma_start(out=xt_t[:], in_=xT[:, r0 : r0 + CW])
                    g1_ps = p_big.tile([P, 1536], F32, space="PSUM", tag="big")
                    for half in range(2):
                        h0_ps = p_small.tile([P, 512], F32, space="PSUM", tag="small")
                        nc.tensor.matmul(
                            out=h0_ps[:],
                            lhsT=we_sb[:],
                            rhs=xt_t[:, half * 512 : (half + 1) * 512],
                            start=True, stop=True,
                        )
                        h0_t = h0t_pool.tile([P, 512], BF16, tag="h0t")
                        nc.scalar.activation(
                            out=h0_t[:], in_=h0_ps[:], func=AF.Relu,
                            bias=be_sb[:, :1],
                        )
                        for sub in range(4):
                            j = half * 4 + sub
                            nc.tensor.matmul(
                                out=g1_ps[:, j * P : (j + 1) * P],
                                lhsT=h0_t[:, sub * P : (sub + 1) * P],
                                rhs=w1_sb[:],
                                start=True, stop=True,
                            )
                    g1_t = g1_pool.tile([P, CW], BF16, tag="g1")
                    nc.vector.tensor_copy(out=g1_t[:], in_=g1_ps[:, 0:CW])
                    # one write: row n=r0+j*128+p  <-  g1_t[p, j*128: ]
                    t_ap = table1[r0 : r0 + CW, :].rearrange(
                        "(j p) d -> p j d", p=P
                    )
                    nc.sync.dma_start(
                        out=t_ap, in_=g1_t[:].rearrange("p (j d) -> p j d", d=D)
                    )

            if stage == 0:
                nc.sync.dma_start(out=dbg[:, :], in_=table1[0:NLOC, :])

            tc.strict_bb_all_engine_barrier()

            # ---------------- aggregation machinery ----------------
            KG = int(os.environ.get("KGMAX", "16"))
            NSWQ = int(os.environ.get("KSWQ", "2"))
            gq_counter = [0]

            def do_group_chunks(table, g, q, ps, started, remaining, cc_dep,
                                bank_started):
                """Gather + S-matmuls for all cells (s in group g, quarter q).
                started/remaining: per-s_local accumulation bookkeeping.
                bank_started: per-PSUM-bank flag — matmul start=True clears
                has_written for the WHOLE bank, so only the first chunk
                touching each bank may carry start=True."""
                ci0, totch, lst = prep.cells[(g, q)]
                if totch == 0:
                    return
                sl_of = np.empty(totch, np.int64)
                for (sl, ciA, nchn) in lst:
                    sl_of[ciA - ci0 : ciA - ci0 + nchn] = sl
                for w0 in range(0, totch, KG):
                    wn = min(KG, totch - w0)
                    m_t = m_pool.tile([P, wn, D], BF16, tag="m")
                    gq_counter[0] += 1
                    gi = nc.gpsimd.dma_gather(
                        m_t[:, :, :],
                        table[q * QR : (q + 1) * QR, :],
                        idx_sb[:, 8 * (ci0 + w0) : 8 * (ci0 + w0 + wn)],
                        wn * P, wn * P, D,
                        queue_num=gq_counter[0] % NSWQ,
                    )
                    if cc_dep is not None:
                        _add_dep_helper(
                            gi.ins, cc_dep.ins, sync=True,
                            reason="gather waits for AllGather block",
                        )
                    for k in range(wn):
                        ci = ci0 + w0 + k
                        sl = int(sl_of[w0 + k])
                        s_t = s_pool.tile([P, W], BF16, tag="s")
                        nc.vector.tensor_scalar(
                            out=s_t[:], in0=iota_bf[:],
                            scalar1=dstl_sb[:, ci : ci + 1],
                            scalar2=sd_sb[:, ci : ci + 1],
                            op0=ALU.is_equal, op1=ALU.mult,
                        )
                        remaining[sl] -= 1
                        bank = sl // 2
                        nc.tensor.matmul(
                            out=ps[:, sl * W : (sl + 1) * W],
                            lhsT=m_t[:, k, :],
                            rhs=s_t[:],
                            start=not bank_started[bank],
                            stop=remaining[sl] == 0,
                            skip_group_check=True,
                        )
                        bank_started[bank] = True
                        started[sl] = True

            # ---------------- Phase L1 (dst-major) + progressive AG -------
            cc_insts = []
            with nc.named_scope("L1"):
                for g in range(NG if stage >= 1 else 0):
                    ps = p_big.tile([P, 1536], F32, space="PSUM", tag="big")
                    started = [False] * G
                    bank_started = [False] * 3
                    remaining = [int(prep.ch_per_super[g * G + sl]) for sl in range(G)]
                    for q in range(NQ):
                        do_group_chunks(table1, g, q, ps, started, remaining,
                                        None, bank_started)
                    # epilogue per super: relu -> h1 stash; W2 -> ag_in
                    for sl in range(G):
                        s = g * G + sl
                        if not started[sl]:
                            continue
                        for half in range(2):
                            t = s * 2 + half
                            nc.scalar.activation(
                                out=h1st[:, t * P : (t + 1) * P],
                                in_=ps[:, sl * W + half * P : sl * W + (half + 1) * P],
                                func=AF.Relu, bias=b1_sb[:, :1],
                            )
                        ps2 = p_small.tile([P, 512], F32, space="PSUM", tag="small")
                        for half in range(2):
                            nc.tensor.matmul(
                                out=ps2[:, half * P : (half + 1) * P],
                                lhsT=h1st[:, (s * 2 + half) * P : (s * 2 + half + 1) * P],
                                rhs=w2_sb[:],
                                start=True, stop=True,
                            )
                        g2_t = g1_pool.tile([P, W], BF16, tag="g2")
                        nc.vector.tensor_copy(out=g2_t[:], in_=ps2[:, 0:W])
                        o_ap = ag_in[s * W : (s + 1) * W, :].rearrange(
                            "(u p) d -> p u d", p=P
                        )
                        nc.sync.dma_start(
                            out=o_ap,
                            in_=g2_t[:].rearrange("p (u d) -> p u d", d=D),
                        )
                    if g in ag_after and stage >= 2:
                        b = ag_after[g]
                        cc = nc.gpsimd.collective_compute(
                            "AllGather",
                            ALU.bypass,
                            replica_groups=[list(range(NC))],
                            ins=[ag_in[b * d.BRL : (b + 1) * d.BRL, :]],
                            outs=[table2[b * QR : (b + 1) * QR, :]],
                        )
                        cc_insts.append(cc)

            if stage == 1:
                nc.sync.dma_start(out=dbg[:, :], in_=ag_in[:, :])
            if stage == 2:
                tc.strict_bb_all_engine_barrier()
                nc.sync.dma_start(out=dbg[:, :], in_=table2[0:NLOC, :])
            if nodep and stage >= 3:
                tc.strict_bb_all_engine_barrier()

            # ---------------- Phase L2 (quarter-major) + fused classifier -
            with nc.named_scope("L2"):
                for q in range(NQ if stage >= 3 else 0):
                    for g in range(NG):
                        ps = p_big.tile([P, 1536], F32, space="PSUM", tag="big")
                        started = [False] * G
                        bank_started = [False] * 3
                        remaining = [int(nch_sq[g * G + sl, q]) for sl in range(G)]
                        do_group_chunks(
                            table2, g, q, ps, started, remaining,
                            None if nodep else cc_insts[q], bank_started,
                        )
                        # merge written supers into acc (contiguous runs)
                        sl = 0
                        while sl < G:
                            if not started[sl]:
                                sl += 1
                                continue
                            sl2 = sl
                            while sl2 + 1 < G and started[sl2 + 1]:
                                sl2 += 1
                            a0 = (g * G + sl) * W
                            a1 = (g * G + sl2 + 1) * W
                            nc.vector.tensor_tensor(
                                out=acc2[:, a0:a1],
                                in0=ps[:, sl * W : (sl2 + 1) * W],
                                in1=acc2[:, a0:a1],
                                op=ALU.add,
                            )
                            sl = sl2 + 1
                        if q == NQ - 1:
                            # classifier for this group's tiles
                            for t in range(g * G * 2, (g + 1) * G * 2):
                                h2_t = h_pool.tile([P, P], BF16, tag="h2")
                                nc.vector.tensor_scalar(
                                    out=h2_t[:],
                                    in0=acc2[:, t * P : (t + 1) * P],
                                    scalar1=b2_sb[:, :1],
                                    scalar2=0.0,
                                    op0=ALU.add, op1=ALU.max,
                                )
                                xl_t = xs_pool.tile([P, P], BF16, tag="xl")
                                nc.sync.dma_start(
                                    out=xl_t[:],
                                    in_=xTloc[:, t * P : (t + 1) * P],
                                )
                                h0_ps = p_small.tile(
                                    [P, 512], F32, space="PSUM", tag="small"
                                )
                                nc.tensor.matmul(
                                    out=h0_ps[:, 0:P], lhsT=we_sb[:], rhs=xl_t[:],
                                    start=True, stop=True,
                                )
                                h0_t = h_pool.tile([P, P], BF16, tag="h0c")
                                nc.scalar.activation(
                                    out=h0_t[:], in_=h0_ps[:, 0:P], func=AF.Relu,
                                    bias=be_sb[:, :1],
                                )
                                o_ps = p_small.tile(
                                    [P, 512], F32, space="PSUM", tag="small"
                                )
                                nc.tensor.matmul(
                                    out=o_ps[:, 0:D_OUT], lhsT=h0_t[:], rhs=wc0_sb[:],
                                    start=True, stop=False,
                                )
                                nc.tensor.matmul(
                                    out=o_ps[:, 0:D_OUT],
                                    lhsT=h1st[:, t * P : (t + 1) * P],
                                    rhs=wc1_sb[:],
                                    start=False, stop=False,
                                )
                                nc.tensor.matmul(
                                    out=o_ps[:, 0:D_OUT], lhsT=h2_t[:], rhs=wc2_sb[:],
                                    start=False, stop=True,
                                )
                                o_t = cls_pool.tile([P, D_OUT], F32, tag="o")
                                nc.vector.tensor_tensor(
                                    out=o_t[:], in0=o_ps[:, 0:D_OUT], in1=bcls_sb[:],
                                    op=ALU.add,
                                )
                                nc.sync.dma_start(
                                    out=out_p[t * P : (t + 1) * P, :], in_=o_t[:]
                                )
    nc.compile()
    return nc


_CACHE = {}


def run(x, edge_index, W_embed, b_embed, W_conv1, b_conv1, W_conv2, b_conv2,
        W_cls, b_cls, dims: Dims, trace=False, tmpdir=None):
    import ml_dtypes
    from concourse.bass_utils import run_bass_kernel_spmd

    import os
    bf16 = np.dtype(ml_dtypes.bfloat16)
    key = (dims.N, os.environ.get("KSTAGE", "3"), os.environ.get("KNODEP", "0"),
           os.environ.get("KGMAX", "16"), os.environ.get("KSWQ", "2"),
           os.environ.get("KSCRATCH", "49152"))
    if key not in _CACHE:
        prep = Prep(np.asarray(edge_index), dims)
        nck = build_kernel(prep)
        _CACHE[key] = (prep, nck)
    prep, nck = _CACHE[key]

    xt_full, xt_loc = prep.make_xt(np.asarray(x, np.float32))
    bcls_t = np.broadcast_to(
        np.asarray(b_cls, np.float32).reshape(1, D_OUT), (P, D_OUT)
    ).copy()

    in_maps = []
    for c in range(NC):
        in_maps.append(
            {
                "xT": xt_full,
                "xTloc": xt_loc[c],
                "idx_all": prep.qidx[c],
                "dstl_all": prep.dstl[c],
                "sd_all": prep.sd[c],
                "W_embed": np.asarray(W_embed, np.float32).astype(bf16),
                "b_embed": np.asarray(b_embed, np.float32).reshape(D, 1),
                "W1": np.asarray(W_conv1, np.float32).astype(bf16),
                "b1": np.asarray(b_conv1, np.float32).reshape(D, 1),
                "W2": np.asarray(W_conv2, np.float32).astype(bf16),
                "b2": np.asarray(b_conv2, np.float32).reshape(D, 1),
                "Wc0": np.asarray(W_cls[0:D, :], np.float32).astype(bf16),
                "Wc1": np.asarray(W_cls[D : 2 * D, :], np.float32).astype(bf16),
                "Wc2": np.asarray(W_cls[2 * D : 3 * D, :], np.float32).astype(bf16),
                "bcls": bcls_t,
            }
        )

    res = run_bass_kernel_spmd(
        nck, in_maps, list(range(NC)), trace=trace, tmpdir=tmpdir
    )

    out = np.empty((dims.N, D_OUT), np.float32)
    for c in range(NC):
        o = res.results[c]["out"]
        m = prep.g_of_p[c] >= 0
        out[prep.g_of_p[c][m]] = o[m]
    return out, res


def kernel(**inputs) -> np.ndarray:
    dims = Dims(100000)
    out, _ = run(
        inputs["x"], inputs["edge_index"], inputs["W_embed"], inputs["b_embed"],
        inputs["W_conv1"], inputs["b_conv1"], inputs["W_conv2"],
        inputs["b_conv2"], inputs["W_cls"], inputs["b_cls"], dims,
    )
    return out
